# revision 16
# baseline (speedup 1.0000x reference)
"""Sliding-window attention (window=128) with attention sinks on 8 Trainium2
cores.

Sharding: tensor-parallel over heads. Core c owns Q heads 4c..4c+3 and KV head
c (GQA group). Each core computes QKV projections for its heads over the full
sequence, RoPE, block-banded sliding-window attention (each 128-query block
attends exactly to its own and the previous 128-key block), and a partial
output projection through its 256 columns of wo. The host sums the 8 partial
outputs and adds wo_b.

Layouts on device (per core):
  - x is passed transposed (xT [HID, S], fp32r-rounded) so the hidden dim is
    the matmul contraction dim on partitions.
  - QKV weights are fused and row-permuted: m-tile 0 = first rotary halves of
    the 4 heads (4x32 rows), m-tile 1 = second halves, m-tile 2 = [k_half1,
    k_half2, v]. This makes RoPE lane-aligned: both rotary halves of a head
    live at the same partitions in two different tiles.
  - Scores per (head, qblock) are one [128q, 256k] PSUM tile covering the
    previous+current key blocks, computed as two accumulating K=32 matmuls
    (one per rotary half). fp32r at N=256 runs at full PE rate.
  - Softmax: DVE adds a {0, -1e6} band mask to the raw scores, ScalarE does
    exp(scale*x) with a fused per-row accumulate (the denominator), then the
    sink term exp(sink) is added and reciprocals are taken for all 4 heads of
    a query block at once. No row-max subtraction: scores here are O(+-5) so
    fp32 exp is exact enough.
  - P is normalized, transposed on the PE (via identity), and used as the
    moving operand of a PV matmul with V in natural [keys, d] layout, giving
    attn^T [d, q] directly -- which is exactly the contraction layout the wo
    matmul needs as its stationary operand.
"""
import sys

sys.path.insert(0, '/opt/trn_rl_repo')
import numpy as np

S = 2048
HID = 2048
D = 64
ROT = 32
NQH = 4            # q heads per core
NCORES = 8
CS = 512           # sequence chunk
NCHUNK = S // CS
QB = 128           # query/key block
NQB = S // QB
KT = HID // 128    # contraction tiles for projections

_CACHE = {}


def _round_fp32r(a: np.ndarray) -> np.ndarray:
    """Round fp32 to the 11-bit-mantissa fp32r format (RNE), low 12 bits 0."""
    b = np.ascontiguousarray(a, dtype=np.float32).view(np.uint32).astype(np.uint64)
    b = (b + 0x7FF + ((b >> 12) & 1)) & 0xFFFFF000
    return b.astype(np.uint32).view(np.float32)


def _build_nc():
    import concourse.mybir as mybir
    import concourse.tile as tile
    from concourse import bacc
    from concourse.masks import make_identity

    F32 = mybir.dt.float32
    F32R = mybir.dt.float32r
    EXP = mybir.ActivationFunctionType.Exp
    MULT = mybir.AluOpType.mult
    ADD = mybir.AluOpType.add

    nc = bacc.Bacc("TRN2", target_bir_lowering=False, debug=False)

    d_xT = nc.dram_tensor("xT", [HID, S], F32R, kind="ExternalInput")
    d_wqkvT = nc.dram_tensor("wqkvT", [HID, 3 * 128], F32R, kind="ExternalInput")
    d_qkvb = nc.dram_tensor("qkvb", [128, 3], F32, kind="ExternalInput")
    d_woT = nc.dram_tensor("woT", [2 * 128, HID], F32R, kind="ExternalInput")
    d_cos4 = nc.dram_tensor("cos4", [128, S], F32, kind="ExternalInput")
    d_sin4 = nc.dram_tensor("sin4", [128, S], F32, kind="ExternalInput")
    d_masks = nc.dram_tensor("masks", [128, 256], F32, kind="ExternalInput")
    d_sinkrep = nc.dram_tensor("sinkrep", [128, NQH], F32, kind="ExternalInput")
    d_out = nc.dram_tensor("out_p", [S, HID], F32, kind="ExternalOutput")

    xT_r = d_xT[:, :].rearrange("(t p) s -> p t s", p=128)

    from contextlib import ExitStack
    with tile.TileContext(nc) as tc, ExitStack() as ctx:
        cpool = ctx.enter_context(tc.tile_pool(name="const", bufs=1))
        kvpool = ctx.enter_context(tc.tile_pool(name="kvpersist", bufs=1))
        xpool = ctx.enter_context(tc.tile_pool(name="xt", bufs=2))
        rawpool = ctx.enter_context(tc.tile_pool(name="raw", bufs=1))
        qrotp = ctx.enter_context(tc.tile_pool(name="qrot", bufs=2))
        rtmp = ctx.enter_context(tc.tile_pool(name="rtmp", bufs=2))
        ktmp = ctx.enter_context(tc.tile_pool(name="ktmp", bufs=2))
        ppool = ctx.enter_context(tc.tile_pool(name="p", bufs=2))
        pmpool = ctx.enter_context(tc.tile_pool(name="pm", bufs=5))
        ptpool = ctx.enter_context(tc.tile_pool(name="pts", bufs=2))
        dpool = ctx.enter_context(tc.tile_pool(name="denom", bufs=2))
        apool = ctx.enter_context(tc.tile_pool(name="attnT", bufs=1))
        opool = ctx.enter_context(tc.tile_pool(name="osb", bufs=3))
        psA = ctx.enter_context(tc.tile_pool(name="psA", bufs=2, space="PSUM"))
        psB = ctx.enter_context(tc.tile_pool(name="psB", bufs=4, space="PSUM"))
        psC = ctx.enter_context(tc.tile_pool(name="psC", bufs=2, space="PSUM"))

        # ---- constants ----
        wq_sb = cpool.tile([128, KT, 3 * 128], F32R)
        nc.sync.dma_start(wq_sb[:], d_wqkvT[:, :].rearrange("(t p) o -> p t o", p=128))
        wo_sb = cpool.tile([128, 2, HID], F32R)
        nc.sync.dma_start(wo_sb[:], d_woT[:, :].rearrange("(t p) h -> p t h", p=128))
        cos_sb = cpool.tile([128, S], F32)
        nc.sync.dma_start(cos_sb[:], d_cos4[:, :])
        sin_sb = cpool.tile([128, S], F32)
        nc.sync.dma_start(sin_sb[:], d_sin4[:, :])
        mask_sb = cpool.tile([128, 256], F32)
        nc.sync.dma_start(mask_sb[:], d_masks[:, :])
        qkvb_sb = cpool.tile([128, 3], F32)
        nc.sync.dma_start(qkvb_sb[:], d_qkvb[:, :])
        sink_sb = cpool.tile([128, NQH], F32)
        nc.sync.dma_start(sink_sb[:], d_sinkrep[:, :])
        esink = cpool.tile([128, NQH], F32)
        nc.scalar.activation(esink[:], sink_sb[:], EXP)
        ident = cpool.tile([128, 128], F32)
        make_identity(nc, ident[:])

        # ---- persistent per-core K/V state ----
        # PE operand base partitions must be in {0, 32, 64}, so K halves are
        # replicated x2 in [64, S] tiles and Q heads are split into two
        # 64-partition groups (heads 0,1 / heads 2,3).
        ka2 = kvpool.tile([64, S], F32R)    # rope'd K half1, replicated x2
        kb2 = kvpool.tile([64, S], F32R)    # rope'd K half2, replicated x2
        vnat = kvpool.tile([128, NQB, D], F32)  # V in [keys, d] layout

        for ci in range(NCHUNK):
            s0 = ci * CS
            ssl = slice(s0, s0 + CS)
            xt = xpool.tile([128, KT, CS], F32R, tag="xt")
            nc.sync.dma_start(xt[:], xT_r[:, :, ssl])

            # QKV projections (fp32r, N=CS)
            raws = []
            for m in range(3):
                ps = psA.tile([128, CS], F32, tag="mmA")
                for t in range(KT):
                    nc.tensor.matmul(
                        ps[:], wq_sb[:, t, m * 128:(m + 1) * 128], xt[:, t, :],
                        start=(t == 0), stop=(t == KT - 1))
                raw = rawpool.tile([128, CS], F32, tag=f"raw{m}")
                nc.vector.tensor_scalar_add(raw[:], ps[:], qkvb_sb[:, m:m + 1])
                raws.append(raw)
            qa_raw, qb_raw, kv_raw = raws

            # K half2 to partitions 0-31 so RoPE is lane-aligned
            kbs = ktmp.tile([32, CS], F32, tag="kbs")
            nc.sync.dma_start(kbs[:], kv_raw[32:64, :])

            # RoPE Q (4 heads stacked as halves at same lanes)
            t1 = rtmp.tile([128, CS], F32, tag="rt1")
            nc.vector.tensor_mul(out=t1[:], in0=qa_raw[:], in1=cos_sb[:, ssl])
            t2 = rtmp.tile([128, CS], F32, tag="rt2")
            nc.vector.tensor_mul(out=t2[:], in0=qb_raw[:], in1=sin_sb[:, ssl])
            qa_rot = qrotp.tile([128, CS], F32R, tag="qar")
            nc.vector.tensor_sub(out=qa_rot[:], in0=t1[:], in1=t2[:])
            t3 = rtmp.tile([128, CS], F32, tag="rt1")
            nc.vector.tensor_mul(out=t3[:], in0=qa_raw[:], in1=sin_sb[:, ssl])
            t4 = rtmp.tile([128, CS], F32, tag="rt2")
            nc.vector.tensor_mul(out=t4[:], in0=qb_raw[:], in1=cos_sb[:, ssl])
            qb_rot = qrotp.tile([128, CS], F32R, tag="qbr")
            nc.vector.tensor_add(out=qb_rot[:], in0=t3[:], in1=t4[:])
            # heads 2,3 shifted down to base 0 for legal matmul operand bases
            qaB = qrotp.tile([64, CS], F32R, tag="qaB")
            nc.sync.dma_start(qaB[:], qa_rot[64:128, :])
            qbB = qrotp.tile([64, CS], F32R, tag="qbB")
            nc.sync.dma_start(qbB[:], qb_rot[64:128, :])

            # RoPE K at partitions 0-31, writing fp32r into ka2/kb2
            u1 = ktmp.tile([32, CS], F32, tag="u1")
            nc.vector.tensor_mul(out=u1[:], in0=kv_raw[0:32, :], in1=cos_sb[0:32, ssl])
            u2 = ktmp.tile([32, CS], F32, tag="u2")
            nc.vector.tensor_mul(out=u2[:], in0=kbs[:], in1=sin_sb[0:32, ssl])
            nc.vector.tensor_sub(out=ka2[0:32, ssl], in0=u1[:], in1=u2[:])
            u3 = ktmp.tile([32, CS], F32, tag="u1")
            nc.vector.tensor_mul(out=u3[:], in0=kv_raw[0:32, :], in1=sin_sb[0:32, ssl])
            u4 = ktmp.tile([32, CS], F32, tag="u2")
            nc.vector.tensor_mul(out=u4[:], in0=kbs[:], in1=cos_sb[0:32, ssl])
            nc.vector.tensor_add(out=kb2[0:32, ssl], in0=u3[:], in1=u4[:])
            nc.sync.dma_start(ka2[32:64, ssl], ka2[0:32, ssl])
            nc.sync.dma_start(kb2[32:64, ssl], kb2[0:32, ssl])

            # V^T -> V natural per key block (PE transpose)
            for j in range(CS // QB):
                kbi = ci * (CS // QB) + j
                vt = psC.tile([128, 128], F32, tag="psC")
                nc.tensor.transpose(
                    vt[:, 0:D], kv_raw[64:128, j * QB:(j + 1) * QB],
                    ident[64:128, 64:128])
                nc.vector.tensor_copy(vnat[:, kbi, :], vt[:, 0:D])

            # ---- attention per query block ----
            for j in range(CS // QB):
                qb = ci * (CS // QB) + j
                q0 = j * QB
                two = qb > 0
                N = 256 if two else 128
                klo = (qb - 1) * QB if two else 0
                dall = dpool.tile([128, NQH], F32, tag="dall")
                pes = []
                for h in range(NQH):
                    hp = slice(32 * (h % 2), 32 * (h % 2) + 32)
                    qa_src = qa_rot if h < 2 else qaB
                    qb_src = qb_rot if h < 2 else qbB
                    sc = psB.tile([128, 256], F32, tag="psB")
                    nc.tensor.matmul(sc[:, :N], qa_src[hp, q0:q0 + QB],
                                     ka2[hp, klo:klo + N], start=True, stop=False)
                    nc.tensor.matmul(sc[:, :N], qb_src[hp, q0:q0 + QB],
                                     kb2[hp, klo:klo + N], start=False, stop=True)
                    smk = ppool.tile([128, 256], F32, tag="smk")
                    nc.vector.tensor_add(out=smk[:, :N], in0=sc[:, :N],
                                         in1=mask_sb[:, 256 - N:])
                    pexp = pmpool.tile([128, 256], F32, tag="pexp")
                    nc.scalar.activation(pexp[:, :N], smk[:, :N], EXP, scale=0.125,
                                         accum_out=dall[:, h:h + 1])
                    pes.append(pexp)
                dal2 = dpool.tile([128, NQH], F32, tag="dal2")
                nc.vector.tensor_add(out=dal2[:], in0=dall[:], in1=esink[:])
                rall = dpool.tile([128, NQH], F32, tag="rall")
                nc.vector.reciprocal(rall[:], dal2[:])

                if j == 0:
                    atT = [apool.tile([128, CS], F32R, tag=f"at{k}",
                                      name=f"atT{k}") for k in (0, 1)]
                pvp = None
                for h in range(NQH):
                    pn = ppool.tile([128, 256], F32, tag="pn")
                    nc.vector.tensor_scalar_mul(pn[:, :N], pes[h][:, :N],
                                                rall[:, h:h + 1])
                    ptp = psB.tile([128, 256], F32, tag="psB")
                    nc.tensor.transpose(ptp[:, 0:QB], pn[:, 0:QB], ident[:])
                    if two:
                        nc.tensor.transpose(ptp[:, QB:256], pn[:, QB:256], ident[:])
                    pts = ptpool.tile([128, 256], F32, tag="pts")
                    if h % 2 == 0:
                        nc.scalar.copy(pts[:, :N], ptp[:, :N])
                    else:
                        nc.vector.tensor_copy(pts[:, :N], ptp[:, :N])
                    if h % 2 == 0:
                        pvp = psC.tile([128, 128], F32, tag="psC")
                    oap = pvp[64 * (h % 2):64 * (h % 2) + 64, :]
                    if two:
                        nc.tensor.matmul(oap, vnat[:, qb - 1, :], pts[:, 0:QB],
                                         start=True, stop=False)
                        nc.tensor.matmul(oap, vnat[:, qb, :], pts[:, QB:256],
                                         start=False, stop=True)
                    else:
                        nc.tensor.matmul(oap, vnat[:, 0, :], pts[:, 0:QB],
                                         start=True, stop=True)
                    if h % 2 == 1:
                        if h == 1:
                            nc.scalar.copy(atT[0][:, q0:q0 + QB], pvp[:])
                        else:
                            nc.vector.tensor_copy(atT[1][:, q0:q0 + QB], pvp[:])

                # wo partial: out[q, :] = attnT[:, q].T @ woT
                for n in range(HID // 512):
                    wop = psA.tile([128, 512], F32, tag="mmA")
                    nc.tensor.matmul(wop[:], atT[0][:, q0:q0 + QB],
                                     wo_sb[:, 0, n * 512:(n + 1) * 512],
                                     start=True, stop=False)
                    nc.tensor.matmul(wop[:], atT[1][:, q0:q0 + QB],
                                     wo_sb[:, 1, n * 512:(n + 1) * 512],
                                     start=False, stop=True)
                    osb = opool.tile([128, 512], F32, tag="osb")
                    if n % 2 == 0:
                        nc.scalar.copy(osb[:], wop[:])
                    else:
                        nc.vector.tensor_copy(osb[:], wop[:])
                    nc.sync.dma_start(
                        d_out[qb * QB:(qb + 1) * QB, n * 512:(n + 1) * 512], osb[:])

    nc.compile()
    return nc


def _prep_inputs(x, cos, sin, wq_w, wq_b, wk_w, wk_b, wv_w, wv_b, wo_w, wo_b,
                 sinks):
    x = np.asarray(x, np.float32)
    cos = np.asarray(cos, np.float32)
    sin = np.asarray(sin, np.float32)
    wq_w = np.asarray(wq_w, np.float32)
    wq_b = np.asarray(wq_b, np.float32)
    wk_w = np.asarray(wk_w, np.float32)
    wk_b = np.asarray(wk_b, np.float32)
    wv_w = np.asarray(wv_w, np.float32)
    wv_b = np.asarray(wv_b, np.float32)
    wo_w = np.asarray(wo_w, np.float32)
    sinks = np.asarray(sinks, np.float32)

    xT = _round_fp32r(x[0].T)                       # [HID, S]
    cos4 = np.ascontiguousarray(np.tile(cos.T, (4, 1)), np.float32)  # [128, S]
    sin4 = np.ascontiguousarray(np.tile(sin.T, (4, 1)), np.float32)
    qi = np.arange(QB)[:, None]
    kj = np.arange(QB)[None, :]
    masks = np.concatenate(
        [np.where(qi <= kj, 0.0, -1e6), np.where(qi >= kj, 0.0, -1e6)],
        axis=1).astype(np.float32)                  # [128, 256] prev|diag additive

    in_maps = []
    for c in range(NCORES):
        rows_a, rows_b = [], []
        ba, bb = [], []
        for jh in range(NQH):
            g = (4 * c + jh) * D
            rows_a.append(wq_w[g:g + ROT])
            rows_b.append(wq_w[g + ROT:g + D])
            ba.append(wq_b[g:g + ROT])
            bb.append(wq_b[g + ROT:g + D])
        kg = c * D
        Wc = np.vstack(rows_a + rows_b +
                       [wk_w[kg:kg + D], wv_w[kg:kg + D]])  # [384, 2048]
        wqkvT = _round_fp32r(Wc.T)
        qkvb = np.stack([
            np.concatenate(ba), np.concatenate(bb),
            np.concatenate([wk_b[kg:kg + D], wv_b[kg:kg + D]]),
        ], axis=1).astype(np.float32)               # [128, 3]
        woT = _round_fp32r(wo_w[:, 256 * c:256 * (c + 1)].T)  # [256, HID]
        sinkrep = np.repeat(sinks[4 * c:4 * c + 4][None, :], 128, 0)
        in_maps.append({
            "xT": xT, "wqkvT": wqkvT, "qkvb": qkvb, "woT": woT,
            "cos4": cos4, "sin4": sin4, "masks": masks,
            "sinkrep": np.ascontiguousarray(sinkrep, np.float32),
        })
    return in_maps


def _run(inputs, trace=False, trace_kwargs=None):
    from concourse.bass_utils import run_bass_kernel_spmd

    if "nc" not in _CACHE:
        _CACHE["nc"] = _build_nc()
    nc = _CACHE["nc"]
    in_maps = _prep_inputs(**inputs)
    res = run_bass_kernel_spmd(
        nc, in_maps, list(range(NCORES)), trace=trace,
        **(trace_kwargs or {}))
    wo_b = np.asarray(inputs["wo_b"], np.float32)
    acc = np.zeros((S, HID), np.float64)
    for r in res.results:
        acc += r["out_p"].astype(np.float64)
    out = (acc + wo_b[None, :].astype(np.float64)).astype(np.float32)
    return out[None], res


def kernel(**inputs) -> np.ndarray:
    out, _ = _run(inputs, trace=False)
    return out


# revision 20
# speedup vs baseline: 1.2909x; 1.2909x over previous
"""Sliding-window attention (window=128) with attention sinks on 8 Trainium2
cores.

Sharding: tensor-parallel over heads. Core c owns Q heads 4c..4c+3 and KV head
c (GQA group). Each core computes QKV projections for its heads over the full
sequence, RoPE, block-banded sliding-window attention (each 128-query block
attends exactly to its own and the previous 128-key block), and a partial
output projection through its 256 columns of wo. The host sums the 8 partial
outputs and adds wo_b.

Key device-side structure (per core):
  - x arrives transposed (xT [HID, S]) and fp32r-rounded so hidden is the
    contraction dim; fused QKV weights are row-permuted so both rotary halves
    of each head live at the same partitions of two m-tiles (lane-aligned
    RoPE), with K/V in the third m-tile.
  - Scores per (head, qblock): one [128q, 256k] PSUM tile over the previous +
    current key block via two accumulating K=32 fp32r matmuls (N=256 runs at
    full PE rate). Additive {0,-1e6} band mask on DVE, exp(0.125*x) with fused
    row-sum on ScalarE, sink term added and reciprocals batched per qblock.
  - P is normalized (fp32r), transposed on the PE, and gathered into per-KEY-
    BLOCK [128k, 256q] tiles (diag half from qblock k, prev half from qblock
    k+1) so the PV matmul runs once per (head, key block) at N=256 fp32r.
    PV output [attn^T diag-part | attn^T prev-part] is assembled into attn^T
    SBUF tiles incrementally (copy + add), and the wo matmul for query block
    q runs one iteration later.
"""
import sys

sys.path.insert(0, '/opt/trn_rl_repo')
import numpy as np

S = 2048
HID = 2048
D = 64
ROT = 32
NQH = 4            # q heads per core
NCORES = 8
CS = 512           # sequence chunk
NCHUNK = S // CS
QB = 128           # query/key block
NQB = S // QB
JPC = CS // QB     # query blocks per chunk
KT = HID // 128    # contraction tiles for projections

_CACHE = {}


def _round_fp32r(a: np.ndarray) -> np.ndarray:
    """Round fp32 to the 11-bit-mantissa fp32r format (RNE), low 12 bits 0."""
    b = np.ascontiguousarray(a, dtype=np.float32).view(np.uint32).astype(np.uint64)
    b = (b + 0x7FF + ((b >> 12) & 1)) & 0xFFFFF000
    return b.astype(np.uint32).view(np.float32)


def _build_nc():
    import concourse.mybir as mybir
    import concourse.tile as tile
    from concourse import bacc
    from concourse.masks import make_identity

    F32 = mybir.dt.float32
    F32R = mybir.dt.float32r
    EXP = mybir.ActivationFunctionType.Exp

    nc = bacc.Bacc("TRN2", target_bir_lowering=False, debug=False)

    d_xT = nc.dram_tensor("xT", [HID, S], F32R, kind="ExternalInput")
    d_wqkvT = nc.dram_tensor("wqkvT", [HID, 3 * 128], F32R, kind="ExternalInput")
    d_qkvb = nc.dram_tensor("qkvb", [128, 3], F32, kind="ExternalInput")
    d_woT = nc.dram_tensor("woT", [2 * 128, HID], F32R, kind="ExternalInput")
    d_cos4 = nc.dram_tensor("cos4", [128, S], F32, kind="ExternalInput")
    d_sin4 = nc.dram_tensor("sin4", [128, S], F32, kind="ExternalInput")
    d_masks = nc.dram_tensor("masks", [128, 256], F32, kind="ExternalInput")
    d_sinkrep = nc.dram_tensor("sinkrep", [128, NQH], F32, kind="ExternalInput")
    d_out = nc.dram_tensor("out_p", [S, HID], F32, kind="ExternalOutput")

    xT_r = d_xT[:, :].rearrange("(t p) s -> p t s", p=128)

    from contextlib import ExitStack
    with tile.TileContext(nc) as tc, ExitStack() as ctx:
        cpool = ctx.enter_context(tc.tile_pool(name="const", bufs=1))
        kvpool = ctx.enter_context(tc.tile_pool(name="kvpersist", bufs=1))
        xpool = ctx.enter_context(tc.tile_pool(name="xt", bufs=2))
        cspool = ctx.enter_context(tc.tile_pool(name="cs", bufs=2))
        rawpool = ctx.enter_context(tc.tile_pool(name="raw", bufs=1))
        qrotp = ctx.enter_context(tc.tile_pool(name="qrot", bufs=2))
        rtmp = ctx.enter_context(tc.tile_pool(name="rtmp", bufs=2))
        ktmp = ctx.enter_context(tc.tile_pool(name="ktmp", bufs=2))
        ppool = ctx.enter_context(tc.tile_pool(name="p", bufs=2))
        pmpool = ctx.enter_context(tc.tile_pool(name="pm", bufs=5))
        ptsp = ctx.enter_context(tc.tile_pool(name="ptsK", bufs=9))
        dpool = ctx.enter_context(tc.tile_pool(name="denom", bufs=2))
        apool = ctx.enter_context(tc.tile_pool(name="attnT", bufs=2))
        opool = ctx.enter_context(tc.tile_pool(name="osb", bufs=2))
        psA = ctx.enter_context(tc.tile_pool(name="psA", bufs=2, space="PSUM"))
        psB = ctx.enter_context(tc.tile_pool(name="psB", bufs=4, space="PSUM"))
        psC = ctx.enter_context(tc.tile_pool(name="psC", bufs=2, space="PSUM"))

        # ---- constants ----
        wq_sb = cpool.tile([128, KT, 3 * 128], F32R)
        nc.sync.dma_start(wq_sb[:], d_wqkvT[:, :].rearrange("(t p) o -> p t o", p=128))
        wo_sb = cpool.tile([128, 2, HID], F32R)
        nc.sync.dma_start(wo_sb[:], d_woT[:, :].rearrange("(t p) h -> p t h", p=128))
        mask_sb = cpool.tile([128, 256], F32)
        nc.sync.dma_start(mask_sb[:], d_masks[:, :])
        qkvb_sb = cpool.tile([128, 3], F32)
        nc.sync.dma_start(qkvb_sb[:], d_qkvb[:, :])
        sink_sb = cpool.tile([128, NQH], F32)
        nc.sync.dma_start(sink_sb[:], d_sinkrep[:, :])
        esink = cpool.tile([128, NQH], F32)
        nc.scalar.activation(esink[:], sink_sb[:], EXP)
        ident = cpool.tile([128, 128], F32)
        make_identity(nc, ident[:])

        # ---- persistent per-core K/V state ----
        # PE operand base partitions must be in {0, 32, 64}, so K halves are
        # replicated x2 in [64, S] tiles and Q heads are split into two
        # 64-partition groups (heads 0,1 / heads 2,3).
        ka2 = kvpool.tile([64, S], F32R)    # rope'd K half1, replicated x2
        kb2 = kvpool.tile([64, S], F32R)    # rope'd K half2, replicated x2
        vnat = kvpool.tile([128, NQB, D], F32)  # V in [keys, d] layout

        pts_hold = {}    # h -> pts tile of key block (qb-1): [diag | prev]
        atT_pairs = {}   # chunk index -> (atT0, atT1) SBUF tiles

        def emit_wo(qb):
            pair = atT_pairs[qb // JPC]
            q0c = (qb % JPC) * QB
            for n in range(HID // 512):
                wop = psA.tile([128, 512], F32, tag="mmA", name="wop")
                nc.tensor.matmul(wop[:], pair[0][:, q0c:q0c + QB],
                                 wo_sb[:, 0, n * 512:(n + 1) * 512],
                                 start=True, stop=False)
                nc.tensor.matmul(wop[:], pair[1][:, q0c:q0c + QB],
                                 wo_sb[:, 1, n * 512:(n + 1) * 512],
                                 start=False, stop=True)
                osb = opool.tile([128, 512], F32, tag="osb", name="osb")
                if n % 2 == 0:
                    nc.scalar.copy(osb[:], wop[:])
                else:
                    nc.vector.tensor_copy(osb[:], wop[:])
                nc.sync.dma_start(
                    d_out[qb * QB:(qb + 1) * QB, n * 512:(n + 1) * 512], osb[:])

        def pv_block(kb, full):
            """PV for key block kb (pts tiles complete); assemble attn^T."""
            n = 256 if full else 128
            pair_d = atT_pairs[kb // JPC]           # atT tiles of qblock kb
            cd = (kb % JPC) * QB
            if full:
                pair_p = atT_pairs[(kb + 1) // JPC]  # atT tiles of qblock kb+1
                cp = ((kb + 1) % JPC) * QB
            for g in (0, 1):
                pvk = psC.tile([128, 256], F32, tag="pvk", name="pvk")
                for h in (2 * g, 2 * g + 1):
                    oap = pvk[64 * (h % 2):64 * (h % 2) + 64, :n]
                    nc.tensor.matmul(oap, vnat[:, kb, :],
                                     pts_hold[h][:, :n], start=True, stop=True)
                # attn^T(qb=kb) += diag contribution (first write for kb==0)
                if kb == 0:
                    nc.scalar.copy(pair_d[g][:, cd:cd + QB], pvk[:, 0:QB])
                else:
                    nc.vector.tensor_add(out=pair_d[g][:, cd:cd + QB],
                                         in0=pair_d[g][:, cd:cd + QB],
                                         in1=pvk[:, 0:QB])
                # attn^T(qb=kb+1) = prev contribution (first write)
                if full:
                    nc.scalar.copy(pair_p[g][:, cp:cp + QB], pvk[:, QB:256])

        for ci in range(NCHUNK):
            s0 = ci * CS
            ssl = slice(s0, s0 + CS)
            xt = xpool.tile([128, KT, CS], F32R, tag="xt")
            nc.sync.dma_start(xt[:], xT_r[:, :, ssl])
            cos_c = cspool.tile([128, CS], F32, tag="cosc")
            nc.sync.dma_start(cos_c[:], d_cos4[:, ssl])
            sin_c = cspool.tile([128, CS], F32, tag="sinc")
            nc.sync.dma_start(sin_c[:], d_sin4[:, ssl])
            atT_pairs[ci] = [apool.tile([128, CS], F32R, tag=f"at{k}",
                                        name=f"atT{k}") for k in (0, 1)]
            atT_pairs.pop(ci - 2, None)

            # QKV projections (fp32r, N=CS)
            raws = []
            for m in range(3):
                ps = psA.tile([128, CS], F32, tag="mmA", name="projp")
                for t in range(KT):
                    nc.tensor.matmul(
                        ps[:], wq_sb[:, t, m * 128:(m + 1) * 128], xt[:, t, :],
                        start=(t == 0), stop=(t == KT - 1))
                raw = rawpool.tile([128, CS], F32, tag=f"raw{m}", name=f"raw{m}")
                nc.vector.tensor_scalar_add(raw[:], ps[:], qkvb_sb[:, m:m + 1])
                raws.append(raw)
            qa_raw, qb_raw, kv_raw = raws

            # K half2 to partitions 0-31 so RoPE is lane-aligned
            kbs = ktmp.tile([32, CS], F32, tag="kbs")
            nc.sync.dma_start(kbs[:], kv_raw[32:64, :])

            # RoPE Q (4 heads stacked as halves at same lanes)
            t1 = rtmp.tile([128, CS], F32, tag="rt1")
            nc.vector.tensor_mul(out=t1[:], in0=qa_raw[:], in1=cos_c[:])
            t2 = rtmp.tile([128, CS], F32, tag="rt2")
            nc.vector.tensor_mul(out=t2[:], in0=qb_raw[:], in1=sin_c[:])
            qa_rot = qrotp.tile([128, CS], F32R, tag="qar")
            nc.vector.tensor_sub(out=qa_rot[:], in0=t1[:], in1=t2[:])
            t3 = rtmp.tile([128, CS], F32, tag="rt1")
            nc.vector.tensor_mul(out=t3[:], in0=qa_raw[:], in1=sin_c[:])
            t4 = rtmp.tile([128, CS], F32, tag="rt2")
            nc.vector.tensor_mul(out=t4[:], in0=qb_raw[:], in1=cos_c[:])
            qb_rot = qrotp.tile([128, CS], F32R, tag="qbr")
            nc.vector.tensor_add(out=qb_rot[:], in0=t3[:], in1=t4[:])
            # heads 2,3 shifted down to base 0 for legal matmul operand bases
            qaB = qrotp.tile([64, CS], F32R, tag="qaB")
            nc.sync.dma_start(qaB[:], qa_rot[64:128, :])
            qbB = qrotp.tile([64, CS], F32R, tag="qbB")
            nc.sync.dma_start(qbB[:], qb_rot[64:128, :])

            # RoPE K at partitions 0-31, writing fp32r into ka2/kb2
            u1 = ktmp.tile([32, CS], F32, tag="u1")
            nc.vector.tensor_mul(out=u1[:], in0=kv_raw[0:32, :], in1=cos_c[0:32, :])
            u2 = ktmp.tile([32, CS], F32, tag="u2")
            nc.vector.tensor_mul(out=u2[:], in0=kbs[:], in1=sin_c[0:32, :])
            nc.vector.tensor_sub(out=ka2[0:32, ssl], in0=u1[:], in1=u2[:])
            u3 = ktmp.tile([32, CS], F32, tag="u1")
            nc.vector.tensor_mul(out=u3[:], in0=kv_raw[0:32, :], in1=sin_c[0:32, :])
            u4 = ktmp.tile([32, CS], F32, tag="u2")
            nc.vector.tensor_mul(out=u4[:], in0=kbs[:], in1=cos_c[0:32, :])
            nc.vector.tensor_add(out=kb2[0:32, ssl], in0=u3[:], in1=u4[:])
            nc.sync.dma_start(ka2[32:64, ssl], ka2[0:32, ssl])
            nc.sync.dma_start(kb2[32:64, ssl], kb2[0:32, ssl])

            # V^T -> V natural per key block (PE transpose, cast to fp32r)
            for j in range(JPC):
                kbi = ci * JPC + j
                vt = psB.tile([128, 128], F32, tag="psB", name="vt")
                nc.tensor.transpose(
                    vt[:, 0:D], kv_raw[64:128, j * QB:(j + 1) * QB],
                    ident[64:128, 64:128])
                nc.vector.tensor_copy(vnat[:, kbi, :], vt[:, 0:D])

            # ---- attention per query block ----
            for j in range(JPC):
                qb = ci * JPC + j
                q0 = j * QB
                two = qb > 0
                N = 256 if two else 128
                klo = (qb - 1) * QB if two else 0
                dall = dpool.tile([128, NQH], F32, tag="dall")
                pes = []
                for h in range(NQH):
                    hp = slice(32 * (h % 2), 32 * (h % 2) + 32)
                    qa_src = qa_rot if h < 2 else qaB
                    qb_src = qb_rot if h < 2 else qbB
                    sc = psB.tile([128, 256], F32, tag="psB", name="sc")
                    nc.tensor.matmul(sc[:, :N], qa_src[hp, q0:q0 + QB],
                                     ka2[hp, klo:klo + N], start=True, stop=False)
                    nc.tensor.matmul(sc[:, :N], qb_src[hp, q0:q0 + QB],
                                     kb2[hp, klo:klo + N], start=False, stop=True)
                    smk = ppool.tile([128, 256], F32, tag="smk")
                    nc.vector.tensor_add(out=smk[:, :N], in0=sc[:, :N],
                                         in1=mask_sb[:, 256 - N:])
                    pexp = pmpool.tile([128, 256], F32, tag="pexp")
                    nc.scalar.activation(pexp[:, :N], smk[:, :N], EXP, scale=0.125,
                                         accum_out=dall[:, h:h + 1])
                    pes.append(pexp)
                dal2 = dpool.tile([128, NQH], F32, tag="dal2")
                nc.vector.tensor_add(out=dal2[:], in0=dall[:], in1=esink[:])
                rall = dpool.tile([128, NQH], F32, tag="rall")
                nc.vector.reciprocal(rall[:], dal2[:])

                new_hold = {}
                for h in range(NQH):
                    pn = ppool.tile([128, 256], F32, tag="pn")
                    nc.vector.tensor_scalar_mul(pn[:, :N], pes[h][:, :N],
                                                rall[:, h:h + 1])
                    ptp = psB.tile([128, 256], F32, tag="psB", name="ptp")
                    nc.tensor.transpose(ptp[:, 0:QB], pn[:, 0:QB], ident[:])
                    if two:
                        nc.tensor.transpose(ptp[:, QB:256], pn[:, QB:256], ident[:])
                    # prev-part of this qblock completes key block qb-1's tile
                    if two:
                        if h % 2 == 0:
                            nc.scalar.copy(pts_hold[h][:, QB:256], ptp[:, 0:QB])
                        else:
                            nc.vector.tensor_copy(pts_hold[h][:, QB:256],
                                                  ptp[:, 0:QB])
                    # diag-part starts key block qb's tile
                    cur = ptsp.tile([128, 256], F32, tag="ptsK", name="ptsK")
                    dsrc = ptp[:, QB:256] if two else ptp[:, 0:QB]
                    if h % 2 == 0:
                        nc.vector.tensor_copy(cur[:, 0:QB], dsrc)
                    else:
                        nc.scalar.copy(cur[:, 0:QB], dsrc)
                    new_hold[h] = cur
                if two:
                    pv_block(qb - 1, full=True)
                pts_hold = new_hold
                if qb >= 2:
                    emit_wo(qb - 2)

        # flush: key block 15 diag-only PV, then final two wo blocks
        pv_block(NQB - 1, full=False)
        emit_wo(NQB - 2)
        emit_wo(NQB - 1)

    nc.compile()
    return nc


def _prep_inputs(x, cos, sin, wq_w, wq_b, wk_w, wk_b, wv_w, wv_b, wo_w, wo_b,
                 sinks):
    x = np.asarray(x, np.float32)
    cos = np.asarray(cos, np.float32)
    sin = np.asarray(sin, np.float32)
    wq_w = np.asarray(wq_w, np.float32)
    wq_b = np.asarray(wq_b, np.float32)
    wk_w = np.asarray(wk_w, np.float32)
    wk_b = np.asarray(wk_b, np.float32)
    wv_w = np.asarray(wv_w, np.float32)
    wv_b = np.asarray(wv_b, np.float32)
    wo_w = np.asarray(wo_w, np.float32)
    sinks = np.asarray(sinks, np.float32)

    xT = _round_fp32r(x[0].T)                       # [HID, S]
    cos4 = np.ascontiguousarray(np.tile(cos.T, (4, 1)), np.float32)  # [128, S]
    sin4 = np.ascontiguousarray(np.tile(sin.T, (4, 1)), np.float32)
    qi = np.arange(QB)[:, None]
    kj = np.arange(QB)[None, :]
    masks = np.concatenate(
        [np.where(qi <= kj, 0.0, -1e6), np.where(qi >= kj, 0.0, -1e6)],
        axis=1).astype(np.float32)                  # [128, 256] prev|diag additive

    in_maps = []
    for c in range(NCORES):
        rows_a, rows_b = [], []
        ba, bb = [], []
        for jh in range(NQH):
            g = (4 * c + jh) * D
            rows_a.append(wq_w[g:g + ROT])
            rows_b.append(wq_w[g + ROT:g + D])
            ba.append(wq_b[g:g + ROT])
            bb.append(wq_b[g + ROT:g + D])
        kg = c * D
        Wc = np.vstack(rows_a + rows_b +
                       [wk_w[kg:kg + D], wv_w[kg:kg + D]])  # [384, 2048]
        wqkvT = _round_fp32r(Wc.T)
        qkvb = np.stack([
            np.concatenate(ba), np.concatenate(bb),
            np.concatenate([wk_b[kg:kg + D], wv_b[kg:kg + D]]),
        ], axis=1).astype(np.float32)               # [128, 3]
        woT = _round_fp32r(wo_w[:, 256 * c:256 * (c + 1)].T)  # [256, HID]
        sinkrep = np.repeat(sinks[4 * c:4 * c + 4][None, :], 128, 0)
        in_maps.append({
            "xT": xT, "wqkvT": wqkvT, "qkvb": qkvb, "woT": woT,
            "cos4": cos4, "sin4": sin4, "masks": masks,
            "sinkrep": np.ascontiguousarray(sinkrep, np.float32),
        })
    return in_maps


def _run(inputs, trace=False, trace_kwargs=None):
    from concourse.bass_utils import run_bass_kernel_spmd

    if "nc" not in _CACHE:
        _CACHE["nc"] = _build_nc()
    nc = _CACHE["nc"]
    in_maps = _prep_inputs(**inputs)
    res = run_bass_kernel_spmd(
        nc, in_maps, list(range(NCORES)), trace=trace,
        **(trace_kwargs or {}))
    wo_b = np.asarray(inputs["wo_b"], np.float32)
    acc = np.zeros((S, HID), np.float64)
    for r in res.results:
        acc += r["out_p"].astype(np.float64)
    out = (acc + wo_b[None, :].astype(np.float64)).astype(np.float32)
    return out[None], res


def kernel(**inputs) -> np.ndarray:
    out, _ = _run(inputs, trace=False)
    return out


# revision 21
# speedup vs baseline: 1.3294x; 1.0298x over previous
"""Sliding-window attention (window=128) with attention sinks on 8 Trainium2
cores.

Sharding: tensor-parallel over heads. Core c owns Q heads 4c..4c+3 and KV head
c (GQA group). Each core computes QKV projections for its heads over the full
sequence, RoPE, block-banded sliding-window attention (each 128-query block
attends exactly to its own and the previous 128-key block), and a partial
output projection through its 256 columns of wo. The host sums the 8 partial
outputs and adds wo_b.

Key device-side structure (per core):
  - x arrives transposed (xT [HID, S]) and fp32r-rounded so hidden is the
    contraction dim; fused QKV weights are row-permuted so both rotary halves
    of each head live at the same partitions of two m-tiles (lane-aligned
    RoPE), with K/V in the third m-tile.
  - Scores per (head, qblock): one [128q, 256k] PSUM tile over the previous +
    current key block via two accumulating K=32 fp32r matmuls (N=256 runs at
    full PE rate). Additive {0,-1e6} band mask on DVE, exp(0.125*x) with fused
    row-sum on ScalarE, sink term added and reciprocals batched per qblock.
  - P is normalized (fp32r), transposed on the PE, and gathered into per-KEY-
    BLOCK [128k, 256q] tiles (diag half from qblock k, prev half from qblock
    k+1) so the PV matmul runs once per (head, key block) at N=256 fp32r.
    PV output [attn^T diag-part | attn^T prev-part] is assembled into attn^T
    SBUF tiles incrementally (copy + add), and the wo matmul for query block
    q runs one iteration later.
"""
import sys

sys.path.insert(0, '/opt/trn_rl_repo')
import numpy as np

S = 2048
HID = 2048
D = 64
ROT = 32
NQH = 4            # q heads per core
NCORES = 8
CS = 512           # sequence chunk
NCHUNK = S // CS
QB = 128           # query/key block
NQB = S // QB
JPC = CS // QB     # query blocks per chunk
KT = HID // 128    # contraction tiles for projections

_CACHE = {}


def _round_fp32r(a: np.ndarray) -> np.ndarray:
    """Round fp32 to the 11-bit-mantissa fp32r format (RNE), low 12 bits 0."""
    b = np.ascontiguousarray(a, dtype=np.float32).view(np.uint32).astype(np.uint64)
    b = (b + 0x7FF + ((b >> 12) & 1)) & 0xFFFFF000
    return b.astype(np.uint32).view(np.float32)


def _build_nc():
    import concourse.mybir as mybir
    import concourse.tile as tile
    from concourse import bacc
    from concourse.masks import make_identity

    F32 = mybir.dt.float32
    F32R = mybir.dt.float32r
    EXP = mybir.ActivationFunctionType.Exp

    nc = bacc.Bacc("TRN2", target_bir_lowering=False, debug=False)

    d_xT = nc.dram_tensor("xT", [HID, S], F32R, kind="ExternalInput")
    d_wqkvT = nc.dram_tensor("wqkvT", [HID, 3 * 128], F32R, kind="ExternalInput")
    d_qkvb = nc.dram_tensor("qkvb", [128, 3], F32, kind="ExternalInput")
    d_woT = nc.dram_tensor("woT", [2 * 128, HID], F32R, kind="ExternalInput")
    d_cos4 = nc.dram_tensor("cos4", [128, S], F32, kind="ExternalInput")
    d_sin4 = nc.dram_tensor("sin4", [128, S], F32, kind="ExternalInput")
    d_masks = nc.dram_tensor("masks", [128, 256], F32, kind="ExternalInput")
    d_sinkrep = nc.dram_tensor("sinkrep", [128, NQH], F32, kind="ExternalInput")
    d_out = nc.dram_tensor("out_p", [S, HID], F32, kind="ExternalOutput")

    xT_r = d_xT[:, :].rearrange("(t p) s -> p t s", p=128)

    from contextlib import ExitStack
    with tile.TileContext(nc) as tc, ExitStack() as ctx:
        cpool = ctx.enter_context(tc.tile_pool(name="const", bufs=1))
        kvpool = ctx.enter_context(tc.tile_pool(name="kvpersist", bufs=1))
        xpool = ctx.enter_context(tc.tile_pool(name="xt", bufs=2))
        cspool = ctx.enter_context(tc.tile_pool(name="cs", bufs=2))
        rawpool = ctx.enter_context(tc.tile_pool(name="raw", bufs=1))
        qrotp = ctx.enter_context(tc.tile_pool(name="qrot", bufs=2))
        rtmp = ctx.enter_context(tc.tile_pool(name="rtmp", bufs=2))
        ktmp = ctx.enter_context(tc.tile_pool(name="ktmp", bufs=2))
        ppool = ctx.enter_context(tc.tile_pool(name="p", bufs=4))
        pmpool = ctx.enter_context(tc.tile_pool(name="pm", bufs=5))
        ptsp = ctx.enter_context(tc.tile_pool(name="ptsK", bufs=9))
        dpool = ctx.enter_context(tc.tile_pool(name="denom", bufs=2))
        apool = ctx.enter_context(tc.tile_pool(name="attnT", bufs=2))
        opool = ctx.enter_context(tc.tile_pool(name="osb", bufs=3))
        psA = ctx.enter_context(tc.tile_pool(name="psA", bufs=2, space="PSUM"))
        psB = ctx.enter_context(tc.tile_pool(name="psB", bufs=4, space="PSUM"))
        psC = ctx.enter_context(tc.tile_pool(name="psC", bufs=2, space="PSUM"))

        # ---- constants ----
        wq_sb = cpool.tile([128, KT, 3 * 128], F32R)
        nc.sync.dma_start(wq_sb[:], d_wqkvT[:, :].rearrange("(t p) o -> p t o", p=128))
        wo_sb = cpool.tile([128, 2, HID], F32R)
        nc.sync.dma_start(wo_sb[:], d_woT[:, :].rearrange("(t p) h -> p t h", p=128))
        mask_sb = cpool.tile([128, 256], F32)
        nc.sync.dma_start(mask_sb[:], d_masks[:, :])
        qkvb_sb = cpool.tile([128, 3], F32)
        nc.sync.dma_start(qkvb_sb[:], d_qkvb[:, :])
        sink_sb = cpool.tile([128, NQH], F32)
        nc.sync.dma_start(sink_sb[:], d_sinkrep[:, :])
        esink = cpool.tile([128, NQH], F32)
        nc.scalar.activation(esink[:], sink_sb[:], EXP)
        ident = cpool.tile([128, 128], F32)
        make_identity(nc, ident[:])

        # ---- persistent per-core K/V state ----
        # PE operand base partitions must be in {0, 32, 64}, so K halves are
        # replicated x2 in [64, S] tiles and Q heads are split into two
        # 64-partition groups (heads 0,1 / heads 2,3).
        k64r = kvpool.tile([128, S], F32R)  # [ka(32);kb(32)] replicated x2
        vnat = kvpool.tile([128, NQB, D], F32)  # V in [keys, d] layout

        pts_hold = {}    # h -> pts tile of key block (qb-1): [diag | prev]
        atT_pairs = {}   # chunk index -> (atT0, atT1) SBUF tiles

        def emit_wo(qb):
            pair = atT_pairs[qb // JPC]
            q0c = (qb % JPC) * QB
            for n in range(HID // 512):
                wop = psA.tile([128, 512], F32, tag="mmA", name="wop")
                nc.tensor.matmul(wop[:], pair[0][:, q0c:q0c + QB],
                                 wo_sb[:, 0, n * 512:(n + 1) * 512],
                                 start=True, stop=False)
                nc.tensor.matmul(wop[:], pair[1][:, q0c:q0c + QB],
                                 wo_sb[:, 1, n * 512:(n + 1) * 512],
                                 start=False, stop=True)
                osb = opool.tile([128, 512], F32, tag="osb", name="osb")
                if n % 2 == 0:
                    nc.scalar.copy(osb[:], wop[:])
                else:
                    nc.vector.tensor_copy(osb[:], wop[:])
                nc.sync.dma_start(
                    d_out[qb * QB:(qb + 1) * QB, n * 512:(n + 1) * 512], osb[:])

        def pv_block(kb, full):
            """PV for key block kb (pts tiles complete); assemble attn^T."""
            n = 256 if full else 128
            pair_d = atT_pairs[kb // JPC]           # atT tiles of qblock kb
            cd = (kb % JPC) * QB
            if full:
                pair_p = atT_pairs[(kb + 1) // JPC]  # atT tiles of qblock kb+1
                cp = ((kb + 1) % JPC) * QB
            for g in (0, 1):
                pvk = psC.tile([128, 256], F32, tag="pvk", name="pvk")
                for h in (2 * g, 2 * g + 1):
                    oap = pvk[64 * (h % 2):64 * (h % 2) + 64, :n]
                    nc.tensor.matmul(oap, vnat[:, kb, :],
                                     pts_hold[h][:, :n], start=True, stop=True)
                # attn^T(qb=kb) += diag contribution (first write for kb==0)
                if kb == 0:
                    nc.scalar.copy(pair_d[g][:, cd:cd + QB], pvk[:, 0:QB])
                else:
                    nc.vector.tensor_add(out=pair_d[g][:, cd:cd + QB],
                                         in0=pair_d[g][:, cd:cd + QB],
                                         in1=pvk[:, 0:QB])
                # attn^T(qb=kb+1) = prev contribution (first write)
                if full:
                    nc.scalar.copy(pair_p[g][:, cp:cp + QB], pvk[:, QB:256])

        for ci in range(NCHUNK):
            s0 = ci * CS
            ssl = slice(s0, s0 + CS)
            xt = xpool.tile([128, KT, CS], F32R, tag="xt")
            nc.sync.dma_start(xt[:], xT_r[:, :, ssl])
            cos_c = cspool.tile([128, CS], F32, tag="cosc")
            nc.sync.dma_start(cos_c[:], d_cos4[:, ssl])
            sin_c = cspool.tile([128, CS], F32, tag="sinc")
            nc.sync.dma_start(sin_c[:], d_sin4[:, ssl])
            atT_pairs[ci] = [apool.tile([128, CS], F32R, tag=f"at{k}",
                                        name=f"atT{k}") for k in (0, 1)]
            atT_pairs.pop(ci - 2, None)

            # QKV projections (fp32r, N=CS)
            raws = []
            for m in range(3):
                ps = psA.tile([128, CS], F32, tag="mmA", name="projp")
                for t in range(KT):
                    nc.tensor.matmul(
                        ps[:], wq_sb[:, t, m * 128:(m + 1) * 128], xt[:, t, :],
                        start=(t == 0), stop=(t == KT - 1))
                raw = rawpool.tile([128, CS], F32, tag=f"raw{m}", name=f"raw{m}")
                nc.vector.tensor_scalar_add(raw[:], ps[:], qkvb_sb[:, m:m + 1])
                raws.append(raw)
            qa_raw, qb_raw, kv_raw = raws

            # K half2 to partitions 0-31 so RoPE is lane-aligned
            kbs = ktmp.tile([32, CS], F32, tag="kbs")
            nc.sync.dma_start(kbs[:], kv_raw[32:64, :])

            # RoPE Q (4 heads stacked as halves at same lanes)
            t1 = rtmp.tile([128, CS], F32, tag="rt1")
            nc.vector.tensor_mul(out=t1[:], in0=qa_raw[:], in1=cos_c[:])
            t2 = rtmp.tile([128, CS], F32, tag="rt2")
            nc.vector.tensor_mul(out=t2[:], in0=qb_raw[:], in1=sin_c[:])
            qa_rot = qrotp.tile([128, CS], F32R, tag="qar")
            nc.vector.tensor_sub(out=qa_rot[:], in0=t1[:], in1=t2[:])
            t3 = rtmp.tile([128, CS], F32, tag="rt1")
            nc.vector.tensor_mul(out=t3[:], in0=qa_raw[:], in1=sin_c[:])
            t4 = rtmp.tile([128, CS], F32, tag="rt2")
            nc.vector.tensor_mul(out=t4[:], in0=qb_raw[:], in1=cos_c[:])
            qb_rot = qrotp.tile([128, CS], F32R, tag="qbr")
            nc.vector.tensor_add(out=qb_rot[:], in0=t3[:], in1=t4[:])
            # per-head-contiguous [a(32);b(32)] layout for single-matmul scores
            q64 = [qrotp.tile([128, CS], F32R, tag=f"q64{g}", name=f"q64{g}")
                   for g in (0, 1)]
            for g in (0, 1):
                for jh in (0, 1):
                    h = 2 * g + jh
                    nc.sync.dma_start(q64[g][64 * jh:64 * jh + 32, :],
                                      qa_rot[32 * h:32 * h + 32, :])
                    nc.sync.dma_start(q64[g][64 * jh + 32:64 * jh + 64, :],
                                      qb_rot[32 * h:32 * h + 32, :])

            # RoPE K at partitions 0-31, writing fp32r into ka2/kb2
            u1 = ktmp.tile([32, CS], F32, tag="u1")
            nc.vector.tensor_mul(out=u1[:], in0=kv_raw[0:32, :], in1=cos_c[0:32, :])
            u2 = ktmp.tile([32, CS], F32, tag="u2")
            nc.vector.tensor_mul(out=u2[:], in0=kbs[:], in1=sin_c[0:32, :])
            nc.vector.tensor_sub(out=k64r[0:32, ssl], in0=u1[:], in1=u2[:])
            u3 = ktmp.tile([32, CS], F32, tag="u1")
            nc.vector.tensor_mul(out=u3[:], in0=kv_raw[0:32, :], in1=sin_c[0:32, :])
            u4 = ktmp.tile([32, CS], F32, tag="u2")
            nc.vector.tensor_mul(out=u4[:], in0=kbs[:], in1=cos_c[0:32, :])
            kbr = ktmp.tile([32, CS], F32R, tag="kbr")
            nc.vector.tensor_add(out=kbr[:], in0=u3[:], in1=u4[:])
            nc.sync.dma_start(k64r[32:64, ssl], kbr[:])
            nc.sync.dma_start(k64r[64:128, ssl], k64r[0:64, ssl])

            # V^T -> V natural per key block (PE transpose, cast to fp32r)
            for j in range(JPC):
                kbi = ci * JPC + j
                vt = psB.tile([128, 128], F32, tag="psB", name="vt")
                nc.tensor.transpose(
                    vt[:, 0:D], kv_raw[64:128, j * QB:(j + 1) * QB],
                    ident[64:128, 64:128])
                nc.vector.tensor_copy(vnat[:, kbi, :], vt[:, 0:D])

            # ---- attention per query block ----
            for j in range(JPC):
                qb = ci * JPC + j
                q0 = j * QB
                two = qb > 0
                N = 256 if two else 128
                klo = (qb - 1) * QB if two else 0
                dall = dpool.tile([128, NQH], F32, tag="dall")
                pes = []
                for h in range(NQH):
                    hp = slice(64 * (h % 2), 64 * (h % 2) + 64)
                    qsrc = q64[h // 2]
                    sc = psB.tile([128, 256], F32, tag="psB", name="sc")
                    nc.tensor.matmul(sc[:, :N], qsrc[hp, q0:q0 + QB],
                                     k64r[hp, klo:klo + N], start=True, stop=True)
                    smk = ppool.tile([128, 256], F32, tag="smk")
                    nc.vector.tensor_add(out=smk[:, :N], in0=sc[:, :N],
                                         in1=mask_sb[:, 256 - N:])
                    pexp = pmpool.tile([128, 256], F32, tag="pexp")
                    nc.scalar.activation(pexp[:, :N], smk[:, :N], EXP, scale=0.125,
                                         accum_out=dall[:, h:h + 1])
                    pes.append(pexp)
                dal2 = dpool.tile([128, NQH], F32, tag="dal2")
                nc.vector.tensor_add(out=dal2[:], in0=dall[:], in1=esink[:])
                rall = dpool.tile([128, NQH], F32, tag="rall")
                nc.vector.reciprocal(rall[:], dal2[:])

                new_hold = {}
                for h in range(NQH):
                    pn = ppool.tile([128, 256], F32, tag="pn")
                    nc.vector.tensor_scalar_mul(pn[:, :N], pes[h][:, :N],
                                                rall[:, h:h + 1])
                    ptp = psB.tile([128, 256], F32, tag="psB", name="ptp")
                    nc.tensor.transpose(ptp[:, 0:QB], pn[:, 0:QB], ident[:])
                    if two:
                        nc.tensor.transpose(ptp[:, QB:256], pn[:, QB:256], ident[:])
                    # prev-part of this qblock completes key block qb-1's tile
                    if two:
                        if h % 2 == 0:
                            nc.scalar.copy(pts_hold[h][:, QB:256], ptp[:, 0:QB])
                        else:
                            nc.vector.tensor_copy(pts_hold[h][:, QB:256],
                                                  ptp[:, 0:QB])
                    # diag-part starts key block qb's tile
                    cur = ptsp.tile([128, 256], F32, tag="ptsK", name="ptsK")
                    dsrc = ptp[:, QB:256] if two else ptp[:, 0:QB]
                    if h % 2 == 0:
                        nc.vector.tensor_copy(cur[:, 0:QB], dsrc)
                    else:
                        nc.scalar.copy(cur[:, 0:QB], dsrc)
                    new_hold[h] = cur
                if two:
                    pv_block(qb - 1, full=True)
                pts_hold = new_hold
                if qb >= 2:
                    emit_wo(qb - 2)

        # flush: key block 15 diag-only PV, then final two wo blocks
        pv_block(NQB - 1, full=False)
        emit_wo(NQB - 2)
        emit_wo(NQB - 1)

    nc.compile()
    return nc


def _prep_inputs(x, cos, sin, wq_w, wq_b, wk_w, wk_b, wv_w, wv_b, wo_w, wo_b,
                 sinks):
    x = np.asarray(x, np.float32)
    cos = np.asarray(cos, np.float32)
    sin = np.asarray(sin, np.float32)
    wq_w = np.asarray(wq_w, np.float32)
    wq_b = np.asarray(wq_b, np.float32)
    wk_w = np.asarray(wk_w, np.float32)
    wk_b = np.asarray(wk_b, np.float32)
    wv_w = np.asarray(wv_w, np.float32)
    wv_b = np.asarray(wv_b, np.float32)
    wo_w = np.asarray(wo_w, np.float32)
    sinks = np.asarray(sinks, np.float32)

    xT = _round_fp32r(x[0].T)                       # [HID, S]
    cos4 = np.ascontiguousarray(np.tile(cos.T, (4, 1)), np.float32)  # [128, S]
    sin4 = np.ascontiguousarray(np.tile(sin.T, (4, 1)), np.float32)
    qi = np.arange(QB)[:, None]
    kj = np.arange(QB)[None, :]
    masks = np.concatenate(
        [np.where(qi <= kj, 0.0, -1e6), np.where(qi >= kj, 0.0, -1e6)],
        axis=1).astype(np.float32)                  # [128, 256] prev|diag additive

    in_maps = []
    for c in range(NCORES):
        rows_a, rows_b = [], []
        ba, bb = [], []
        for jh in range(NQH):
            g = (4 * c + jh) * D
            rows_a.append(wq_w[g:g + ROT])
            rows_b.append(wq_w[g + ROT:g + D])
            ba.append(wq_b[g:g + ROT])
            bb.append(wq_b[g + ROT:g + D])
        kg = c * D
        Wc = np.vstack(rows_a + rows_b +
                       [wk_w[kg:kg + D], wv_w[kg:kg + D]])  # [384, 2048]
        wqkvT = _round_fp32r(Wc.T)
        qkvb = np.stack([
            np.concatenate(ba), np.concatenate(bb),
            np.concatenate([wk_b[kg:kg + D], wv_b[kg:kg + D]]),
        ], axis=1).astype(np.float32)               # [128, 3]
        woT = _round_fp32r(wo_w[:, 256 * c:256 * (c + 1)].T)  # [256, HID]
        sinkrep = np.repeat(sinks[4 * c:4 * c + 4][None, :], 128, 0)
        in_maps.append({
            "xT": xT, "wqkvT": wqkvT, "qkvb": qkvb, "woT": woT,
            "cos4": cos4, "sin4": sin4, "masks": masks,
            "sinkrep": np.ascontiguousarray(sinkrep, np.float32),
        })
    return in_maps


def _run(inputs, trace=False, trace_kwargs=None):
    from concourse.bass_utils import run_bass_kernel_spmd

    if "nc" not in _CACHE:
        _CACHE["nc"] = _build_nc()
    nc = _CACHE["nc"]
    in_maps = _prep_inputs(**inputs)
    res = run_bass_kernel_spmd(
        nc, in_maps, list(range(NCORES)), trace=trace,
        **(trace_kwargs or {}))
    wo_b = np.asarray(inputs["wo_b"], np.float32)
    acc = np.zeros((S, HID), np.float64)
    for r in res.results:
        acc += r["out_p"].astype(np.float64)
    out = (acc + wo_b[None, :].astype(np.float64)).astype(np.float32)
    return out[None], res


def kernel(**inputs) -> np.ndarray:
    out, _ = _run(inputs, trace=False)
    return out


# revision 23
# speedup vs baseline: 1.3329x; 1.0026x over previous
"""Sliding-window attention (window=128) with attention sinks on 8 Trainium2
cores.

Sharding: tensor-parallel over heads. Core c owns Q heads 4c..4c+3 and KV head
c (GQA group). Each core computes QKV projections for its heads over the full
sequence, RoPE, block-banded sliding-window attention (each 128-query block
attends exactly to its own and the previous 128-key block), and a partial
output projection through its 256 columns of wo. The host sums the 8 partial
outputs and adds wo_b.

Key device-side structure (per core):
  - x arrives transposed (xT [HID, S]) and fp32r-rounded so hidden is the
    contraction dim; fused QKV weights are row-permuted so both rotary halves
    of each head live at the same partitions of two m-tiles (lane-aligned
    RoPE), with K/V in the third m-tile.
  - Scores per (head, qblock): one [128q, 256k] PSUM tile over the previous +
    current key block via two accumulating K=32 fp32r matmuls (N=256 runs at
    full PE rate). Additive {0,-1e6} band mask on DVE, exp(0.125*x) with fused
    row-sum on ScalarE, sink term added and reciprocals batched per qblock.
  - P is normalized (fp32r), transposed on the PE, and gathered into per-KEY-
    BLOCK [128k, 256q] tiles (diag half from qblock k, prev half from qblock
    k+1) so the PV matmul runs once per (head, key block) at N=256 fp32r.
    PV output [attn^T diag-part | attn^T prev-part] is assembled into attn^T
    SBUF tiles incrementally (copy + add), and the wo matmul for query block
    q runs one iteration later.
"""
import sys

sys.path.insert(0, '/opt/trn_rl_repo')
import numpy as np

S = 2048
HID = 2048
D = 64
ROT = 32
NQH = 4            # q heads per core
NCORES = 8
CS = 512           # sequence chunk
NCHUNK = S // CS
QB = 128           # query/key block
NQB = S // QB
JPC = CS // QB     # query blocks per chunk
KT = HID // 128    # contraction tiles for projections

_CACHE = {}


def _round_fp32r(a: np.ndarray) -> np.ndarray:
    """Round fp32 to the 11-bit-mantissa fp32r format (RNE), low 12 bits 0."""
    b = np.ascontiguousarray(a, dtype=np.float32).view(np.uint32).astype(np.uint64)
    b = (b + 0x7FF + ((b >> 12) & 1)) & 0xFFFFF000
    return b.astype(np.uint32).view(np.float32)


def _build_nc():
    import concourse.mybir as mybir
    import concourse.tile as tile
    from concourse import bacc
    from concourse.masks import make_identity

    F32 = mybir.dt.float32
    F32R = mybir.dt.float32r
    EXP = mybir.ActivationFunctionType.Exp

    nc = bacc.Bacc("TRN2", target_bir_lowering=False, debug=False)

    d_xT = nc.dram_tensor("xT", [HID, S], F32R, kind="ExternalInput")
    d_wqkvT = nc.dram_tensor("wqkvT", [HID, 3 * 128], F32R, kind="ExternalInput")
    d_qkvb = nc.dram_tensor("qkvb", [128, 3], F32, kind="ExternalInput")
    d_woT = nc.dram_tensor("woT", [2 * 128, HID], F32R, kind="ExternalInput")
    d_cos4 = nc.dram_tensor("cos4", [128, S], F32, kind="ExternalInput")
    d_sin4 = nc.dram_tensor("sin4", [128, S], F32, kind="ExternalInput")
    d_masks = nc.dram_tensor("masks", [128, 256], F32, kind="ExternalInput")
    d_sinkrep = nc.dram_tensor("sinkrep", [128, NQH], F32, kind="ExternalInput")
    d_out = nc.dram_tensor("out_p", [S, HID], F32, kind="ExternalOutput")

    xT_r = d_xT[:, :].rearrange("(t p) s -> p t s", p=128)

    from contextlib import ExitStack
    with tile.TileContext(nc) as tc, ExitStack() as ctx:
        cpool = ctx.enter_context(tc.tile_pool(name="const", bufs=1))
        kvpool = ctx.enter_context(tc.tile_pool(name="kvpersist", bufs=1))
        xpool = ctx.enter_context(tc.tile_pool(name="xt", bufs=2))
        cspool = ctx.enter_context(tc.tile_pool(name="cs", bufs=2))
        rawpool = ctx.enter_context(tc.tile_pool(name="raw", bufs=1))
        qrotp = ctx.enter_context(tc.tile_pool(name="qrot", bufs=2))
        rtmp = ctx.enter_context(tc.tile_pool(name="rtmp", bufs=2))
        ktmp = ctx.enter_context(tc.tile_pool(name="ktmp", bufs=2))
        ppool = ctx.enter_context(tc.tile_pool(name="p", bufs=4))
        pmpool = ctx.enter_context(tc.tile_pool(name="pm", bufs=5))
        ptsp = ctx.enter_context(tc.tile_pool(name="ptsK", bufs=9))
        dpool = ctx.enter_context(tc.tile_pool(name="denom", bufs=2))
        apool = ctx.enter_context(tc.tile_pool(name="attnT", bufs=2))
        opool = ctx.enter_context(tc.tile_pool(name="osb", bufs=3))
        psA = ctx.enter_context(tc.tile_pool(name="psA", bufs=3, space="PSUM"))
        psB = ctx.enter_context(tc.tile_pool(name="psB", bufs=3, space="PSUM"))
        psC = ctx.enter_context(tc.tile_pool(name="psC", bufs=2, space="PSUM"))

        # ---- constants ----
        wq_sb = cpool.tile([128, KT, 3 * 128], F32R)
        nc.sync.dma_start(wq_sb[:], d_wqkvT[:, :].rearrange("(t p) o -> p t o", p=128))
        wo_sb = cpool.tile([128, 2, HID], F32R)
        nc.sync.dma_start(wo_sb[:], d_woT[:, :].rearrange("(t p) h -> p t h", p=128))
        mask_sb = cpool.tile([128, 256], F32)
        nc.sync.dma_start(mask_sb[:], d_masks[:, :])
        qkvb_sb = cpool.tile([128, 3], F32)
        nc.sync.dma_start(qkvb_sb[:], d_qkvb[:, :])
        sink_sb = cpool.tile([128, NQH], F32)
        nc.sync.dma_start(sink_sb[:], d_sinkrep[:, :])
        esink = cpool.tile([128, NQH], F32)
        nc.scalar.activation(esink[:], sink_sb[:], EXP)
        ident = cpool.tile([128, 128], F32)
        make_identity(nc, ident[:])

        # ---- persistent per-core K/V state ----
        # PE operand base partitions must be in {0, 32, 64}, so K halves are
        # replicated x2 in [64, S] tiles and Q heads are split into two
        # 64-partition groups (heads 0,1 / heads 2,3).
        k64r = kvpool.tile([128, S], F32R)  # [ka(32);kb(32)] replicated x2
        vnat = kvpool.tile([128, NQB, D], F32)  # V in [keys, d] layout

        pts_hold = {}    # h -> pts tile of key block (qb-1): [diag | prev]
        atT_pairs = {}   # chunk index -> (atT0, atT1) SBUF tiles

        def emit_wo(qb):
            pair = atT_pairs[qb // JPC]
            q0c = (qb % JPC) * QB
            for n in range(HID // 512):
                wop = psA.tile([128, 512], F32, tag="mmA", name="wop")
                nc.tensor.matmul(wop[:], pair[0][:, q0c:q0c + QB],
                                 wo_sb[:, 0, n * 512:(n + 1) * 512],
                                 start=True, stop=False)
                nc.tensor.matmul(wop[:], pair[1][:, q0c:q0c + QB],
                                 wo_sb[:, 1, n * 512:(n + 1) * 512],
                                 start=False, stop=True)
                osb = opool.tile([128, 512], F32, tag="osb", name="osb")
                if n % 2 == 0:
                    nc.scalar.copy(osb[:], wop[:])
                else:
                    nc.vector.tensor_copy(osb[:], wop[:])
                nc.sync.dma_start(
                    d_out[qb * QB:(qb + 1) * QB, n * 512:(n + 1) * 512], osb[:])

        def pv_block(kb, full):
            """PV for key block kb (pts tiles complete); assemble attn^T."""
            n = 256 if full else 128
            pair_d = atT_pairs[kb // JPC]           # atT tiles of qblock kb
            cd = (kb % JPC) * QB
            if full:
                pair_p = atT_pairs[(kb + 1) // JPC]  # atT tiles of qblock kb+1
                cp = ((kb + 1) % JPC) * QB
            for g in (0, 1):
                pvk = psC.tile([128, 256], F32, tag="pvk", name="pvk")
                for h in (2 * g, 2 * g + 1):
                    oap = pvk[64 * (h % 2):64 * (h % 2) + 64, :n]
                    nc.tensor.matmul(oap, vnat[:, kb, :],
                                     pts_hold[h][:, :n], start=True, stop=True)
                # attn^T(qb=kb) += diag contribution (first write for kb==0)
                if kb == 0:
                    nc.scalar.copy(pair_d[g][:, cd:cd + QB], pvk[:, 0:QB])
                else:
                    nc.vector.tensor_add(out=pair_d[g][:, cd:cd + QB],
                                         in0=pair_d[g][:, cd:cd + QB],
                                         in1=pvk[:, 0:QB])
                # attn^T(qb=kb+1) = prev contribution (first write)
                if full:
                    nc.scalar.copy(pair_p[g][:, cp:cp + QB], pvk[:, QB:256])

        for ci in range(NCHUNK):
            s0 = ci * CS
            ssl = slice(s0, s0 + CS)
            xt = xpool.tile([128, KT, CS], F32R, tag="xt")
            nc.sync.dma_start(xt[:], xT_r[:, :, ssl])
            cos_c = cspool.tile([128, CS], F32, tag="cosc")
            nc.sync.dma_start(cos_c[:], d_cos4[:, ssl])
            sin_c = cspool.tile([128, CS], F32, tag="sinc")
            nc.sync.dma_start(sin_c[:], d_sin4[:, ssl])
            atT_pairs[ci] = [apool.tile([128, CS], F32R, tag=f"at{k}",
                                        name=f"atT{k}") for k in (0, 1)]
            atT_pairs.pop(ci - 2, None)

            # QKV projections (fp32r, N=CS)
            raws = []
            for m in range(3):
                ps = psA.tile([128, CS], F32, tag="mmA", name="projp")
                for t in range(KT):
                    nc.tensor.matmul(
                        ps[:], wq_sb[:, t, m * 128:(m + 1) * 128], xt[:, t, :],
                        start=(t == 0), stop=(t == KT - 1))
                raw = rawpool.tile([128, CS], F32, tag=f"raw{m}", name=f"raw{m}")
                nc.vector.tensor_scalar_add(raw[:], ps[:], qkvb_sb[:, m:m + 1])
                raws.append(raw)
            qa_raw, qb_raw, kv_raw = raws

            # K half2 to partitions 0-31 so RoPE is lane-aligned
            kbs = ktmp.tile([32, CS], F32, tag="kbs")
            nc.sync.dma_start(kbs[:], kv_raw[32:64, :])

            # RoPE Q (4 heads stacked as halves at same lanes)
            t1 = rtmp.tile([128, CS], F32, tag="rt1")
            nc.vector.tensor_mul(out=t1[:], in0=qa_raw[:], in1=cos_c[:])
            t2 = rtmp.tile([128, CS], F32, tag="rt2")
            nc.vector.tensor_mul(out=t2[:], in0=qb_raw[:], in1=sin_c[:])
            qa_rot = qrotp.tile([128, CS], F32R, tag="qar")
            nc.vector.tensor_sub(out=qa_rot[:], in0=t1[:], in1=t2[:])
            t3 = rtmp.tile([128, CS], F32, tag="rt1")
            nc.vector.tensor_mul(out=t3[:], in0=qa_raw[:], in1=sin_c[:])
            t4 = rtmp.tile([128, CS], F32, tag="rt2")
            nc.vector.tensor_mul(out=t4[:], in0=qb_raw[:], in1=cos_c[:])
            qb_rot = qrotp.tile([128, CS], F32R, tag="qbr")
            nc.vector.tensor_add(out=qb_rot[:], in0=t3[:], in1=t4[:])
            # per-head-contiguous [a(32);b(32)] layout for single-matmul scores
            q64 = [qrotp.tile([128, CS], F32R, tag=f"q64{g}", name=f"q64{g}")
                   for g in (0, 1)]
            for g in (0, 1):
                for jh in (0, 1):
                    h = 2 * g + jh
                    nc.sync.dma_start(q64[g][64 * jh:64 * jh + 32, :],
                                      qa_rot[32 * h:32 * h + 32, :])
                    nc.sync.dma_start(q64[g][64 * jh + 32:64 * jh + 64, :],
                                      qb_rot[32 * h:32 * h + 32, :])

            # RoPE K at partitions 0-31, writing fp32r into ka2/kb2
            u1 = ktmp.tile([32, CS], F32, tag="u1")
            nc.vector.tensor_mul(out=u1[:], in0=kv_raw[0:32, :], in1=cos_c[0:32, :])
            u2 = ktmp.tile([32, CS], F32, tag="u2")
            nc.vector.tensor_mul(out=u2[:], in0=kbs[:], in1=sin_c[0:32, :])
            nc.vector.tensor_sub(out=k64r[0:32, ssl], in0=u1[:], in1=u2[:])
            u3 = ktmp.tile([32, CS], F32, tag="u1")
            nc.vector.tensor_mul(out=u3[:], in0=kv_raw[0:32, :], in1=sin_c[0:32, :])
            u4 = ktmp.tile([32, CS], F32, tag="u2")
            nc.vector.tensor_mul(out=u4[:], in0=kbs[:], in1=cos_c[0:32, :])
            kbr = ktmp.tile([32, CS], F32R, tag="kbr")
            nc.vector.tensor_add(out=kbr[:], in0=u3[:], in1=u4[:])
            nc.sync.dma_start(k64r[32:64, ssl], kbr[:])
            nc.sync.dma_start(k64r[64:128, ssl], k64r[0:64, ssl])

            # V^T -> V natural per key block (PE transpose, cast to fp32r)
            for j in range(JPC):
                kbi = ci * JPC + j
                vt = psB.tile([128, 128], F32, tag="psB", name="vt")
                nc.tensor.transpose(
                    vt[:, 0:D], kv_raw[64:128, j * QB:(j + 1) * QB],
                    ident[64:128, 64:128])
                nc.vector.tensor_copy(vnat[:, kbi, :], vt[:, 0:D])

            # ---- attention per query block ----
            for j in range(JPC):
                qb = ci * JPC + j
                q0 = j * QB
                two = qb > 0
                N = 256 if two else 128
                klo = (qb - 1) * QB if two else 0
                dall = dpool.tile([128, NQH], F32, tag="dall")
                pes = []
                for h in range(NQH):
                    hp = slice(64 * (h % 2), 64 * (h % 2) + 64)
                    qsrc = q64[h // 2]
                    sc = psB.tile([128, 256], F32, tag="psB", name="sc")
                    nc.tensor.matmul(sc[:, :N], qsrc[hp, q0:q0 + QB],
                                     k64r[hp, klo:klo + N], start=True, stop=True)
                    smk = ppool.tile([128, 256], F32, tag="smk")
                    nc.vector.tensor_add(out=smk[:, :N], in0=sc[:, :N],
                                         in1=mask_sb[:, 256 - N:])
                    pexp = pmpool.tile([128, 256], F32, tag="pexp")
                    nc.scalar.activation(pexp[:, :N], smk[:, :N], EXP, scale=0.125,
                                         accum_out=dall[:, h:h + 1])
                    pes.append(pexp)
                dal2 = dpool.tile([128, NQH], F32, tag="dal2")
                nc.vector.tensor_add(out=dal2[:], in0=dall[:], in1=esink[:])
                rall = dpool.tile([128, NQH], F32, tag="rall")
                nc.vector.reciprocal(rall[:], dal2[:])

                new_hold = {}
                for h in range(NQH):
                    pn = ppool.tile([128, 256], F32, tag="pn")
                    nc.vector.tensor_scalar_mul(pn[:, :N], pes[h][:, :N],
                                                rall[:, h:h + 1])
                    ptp = psB.tile([128, 256], F32, tag="psB", name="ptp")
                    nc.tensor.transpose(ptp[:, 0:QB], pn[:, 0:QB], ident[:])
                    if two:
                        nc.tensor.transpose(ptp[:, QB:256], pn[:, QB:256], ident[:])
                    # prev-part of this qblock completes key block qb-1's tile
                    if two:
                        if h % 2 == 0:
                            nc.scalar.copy(pts_hold[h][:, QB:256], ptp[:, 0:QB])
                        else:
                            nc.vector.tensor_copy(pts_hold[h][:, QB:256],
                                                  ptp[:, 0:QB])
                    # diag-part starts key block qb's tile
                    cur = ptsp.tile([128, 256], F32, tag="ptsK", name="ptsK")
                    dsrc = ptp[:, QB:256] if two else ptp[:, 0:QB]
                    if h % 2 == 0:
                        nc.vector.tensor_copy(cur[:, 0:QB], dsrc)
                    else:
                        nc.scalar.copy(cur[:, 0:QB], dsrc)
                    new_hold[h] = cur
                if two:
                    pv_block(qb - 1, full=True)
                pts_hold = new_hold
                if qb >= 2:
                    emit_wo(qb - 2)

        # flush: key block 15 diag-only PV, then final two wo blocks
        pv_block(NQB - 1, full=False)
        emit_wo(NQB - 2)
        emit_wo(NQB - 1)

    nc.compile()
    return nc


def _prep_inputs(x, cos, sin, wq_w, wq_b, wk_w, wk_b, wv_w, wv_b, wo_w, wo_b,
                 sinks):
    x = np.asarray(x, np.float32)
    cos = np.asarray(cos, np.float32)
    sin = np.asarray(sin, np.float32)
    wq_w = np.asarray(wq_w, np.float32)
    wq_b = np.asarray(wq_b, np.float32)
    wk_w = np.asarray(wk_w, np.float32)
    wk_b = np.asarray(wk_b, np.float32)
    wv_w = np.asarray(wv_w, np.float32)
    wv_b = np.asarray(wv_b, np.float32)
    wo_w = np.asarray(wo_w, np.float32)
    sinks = np.asarray(sinks, np.float32)

    xT = _round_fp32r(x[0].T)                       # [HID, S]
    cos4 = np.ascontiguousarray(np.tile(cos.T, (4, 1)), np.float32)  # [128, S]
    sin4 = np.ascontiguousarray(np.tile(sin.T, (4, 1)), np.float32)
    qi = np.arange(QB)[:, None]
    kj = np.arange(QB)[None, :]
    masks = np.concatenate(
        [np.where(qi <= kj, 0.0, -1e6), np.where(qi >= kj, 0.0, -1e6)],
        axis=1).astype(np.float32)                  # [128, 256] prev|diag additive

    in_maps = []
    for c in range(NCORES):
        rows_a, rows_b = [], []
        ba, bb = [], []
        for jh in range(NQH):
            g = (4 * c + jh) * D
            rows_a.append(wq_w[g:g + ROT])
            rows_b.append(wq_w[g + ROT:g + D])
            ba.append(wq_b[g:g + ROT])
            bb.append(wq_b[g + ROT:g + D])
        kg = c * D
        Wc = np.vstack(rows_a + rows_b +
                       [wk_w[kg:kg + D], wv_w[kg:kg + D]])  # [384, 2048]
        wqkvT = _round_fp32r(Wc.T)
        qkvb = np.stack([
            np.concatenate(ba), np.concatenate(bb),
            np.concatenate([wk_b[kg:kg + D], wv_b[kg:kg + D]]),
        ], axis=1).astype(np.float32)               # [128, 3]
        woT = _round_fp32r(wo_w[:, 256 * c:256 * (c + 1)].T)  # [256, HID]
        sinkrep = np.repeat(sinks[4 * c:4 * c + 4][None, :], 128, 0)
        in_maps.append({
            "xT": xT, "wqkvT": wqkvT, "qkvb": qkvb, "woT": woT,
            "cos4": cos4, "sin4": sin4, "masks": masks,
            "sinkrep": np.ascontiguousarray(sinkrep, np.float32),
        })
    return in_maps


def _run(inputs, trace=False, trace_kwargs=None):
    from concourse.bass_utils import run_bass_kernel_spmd

    if "nc" not in _CACHE:
        _CACHE["nc"] = _build_nc()
    nc = _CACHE["nc"]
    in_maps = _prep_inputs(**inputs)
    res = run_bass_kernel_spmd(
        nc, in_maps, list(range(NCORES)), trace=trace,
        **(trace_kwargs or {}))
    wo_b = np.asarray(inputs["wo_b"], np.float32)
    acc = np.zeros((S, HID), np.float64)
    for r in res.results:
        acc += r["out_p"].astype(np.float64)
    out = (acc + wo_b[None, :].astype(np.float64)).astype(np.float32)
    return out[None], res


def kernel(**inputs) -> np.ndarray:
    out, _ = _run(inputs, trace=False)
    return out
